# revision 2
# baseline (speedup 1.0000x reference)
"""Trainium2 kernel for nn_DeepLinearTimeSeries.

The reference network is a 400-layer *linear* residual MLP: every step is
x <- x @ (W_i^T) [+ 0.1 * carry], with no nonlinearities anywhere. The whole
stack therefore collapses algebraically to a single matrix:

    out = x @ m,   m = T_enc @ T_temp @ T_dec @ W_out^T  (64 x 1)

where each block's transfer matrix is the product of its per-layer factors
(W_i^T + 0.1*I), with the first two layers of the encoder/temporal blocks
handled per the reference's carry pattern (T = W0^T W1^T + 0.1 I).

We fold the 400 64x64 factors on the host (trivial FLOPs, same f32
arithmetic regime as the reference), then run the remaining memory-bound
pass y = x @ m on 8 NeuronCores, data-parallel over the batch dim
(sharding_hint). Per core: x shard [32768, 64] -> y [32768].

Device kernel (raw Bass, no Tile): x is shipped bf16 (well inside the 2e-2
tolerance; the dot is accumulated in fp32 PSUM) so the HBM stream is 4 MiB
per core instead of 8. The dot itself runs on the *tensor* engine: the host
pre-packs x so the hidden dim lives on partitions -- partition p = 8*j + d
holds dim (8g+d) of token-slot j at free offset (t*4096 + g*512 + c), token
id = t*8192 + c*16 + j. A [128 x 16] stationary slice of mm (m replicated
per dim-group g) then turns each 512-column matmul into 16-token-parallel
multiply+reduce, accumulating the g=0..7 passes into one PSUM tile
[16, 512] per t. 32 matmuls total (~220 ns each) chase 8 x 512 KiB chunk
DMAs that are split across both HWDGE rings (sync + scalar) so per-DMA
completion latency hides under the other ring's data. DVE drains each
finished PSUM tile to SBUF; one 128 KiB DMA returns y.
"""

import numpy as np
from ml_dtypes import bfloat16

import concourse.bass as bass
import concourse.mybir as mybir
from concourse.bass_utils import run_bass_kernel_spmd

# Problem constants (hardcoded per harness contract).
B, S, H = 128, 2048, 64
N_CORES = 8
RW = np.float32(0.1)
ROWS = B * S // N_CORES          # 32768 tokens per core
P = 128                          # SBUF partitions
NG = 8                           # dim groups (8 dims each)
D = H // NG                      # 8 dims per group
J = P // D                       # 16 token-slots per column
C = 512                          # columns per tile (= PSUM bank free size)
NT = ROWS // (J * C)             # 4 PSUM tiles
NCHUNK = 8
FREE = NT * NG * C // NCHUNK     # 2048 free elems per partition per chunk
PASS_PER_CHUNK = FREE // C       # 4 matmul passes per chunk
FP32 = mybir.dt.float32
BF16 = mybir.dt.bfloat16

# Extra kwargs for run_bass_kernel_spmd (test harness sets these for tracing).
RUN_KWARGS: dict = {}


def _collapse_weights(W_enc, W_temp, W_dec, W_out):
    """Fold the full linear stack into a single [H, 1] f32 matrix."""
    eye = np.eye(H, dtype=np.float32)

    def block_mat(Ws):
        # x1 = x0 W0^T ; x2 = x1 W1^T + 0.1 x0 ; then x <- x (Wi^T + 0.1 I)
        T = Ws[0].T @ Ws[1].T + RW * eye
        for Wi in Ws[2:]:
            T = T @ (Wi.T + RW * eye)
        return T

    M = block_mat(W_enc) @ block_mat(W_temp)
    for Wd in W_dec:
        M = M @ (Wd.T + RW * eye)
    return (M @ W_out.T).astype(np.float32)  # [H, 1]


def _pack_shard(x_shard):
    """[32768, 64] f32 -> [128, 16384] bf16, p=(j,d), f=(t,g,c)."""
    xs = x_shard.reshape(NT, C, J, NG, D)           # [t, c, j, g, d]
    return np.ascontiguousarray(
        xs.transpose(2, 4, 0, 3, 1).reshape(P, NT * NG * C)
    ).astype(bfloat16)


def _pack_mm(m):
    """[H,1] f32 -> [128, NG*J] bf16 stationary stack (one slice per g)."""
    mm = np.zeros((P, NG * J), np.float32)
    for g in range(NG):
        for j in range(J):
            mm[D * j : D * j + D, g * J + j] = m[D * g : D * g + D, 0]
    return mm.astype(bfloat16)


def _build_bass():
    nc = bass.Bass()
    x = nc.dram_tensor("x", [P, NT * NG * C], BF16, kind="ExternalInput")
    mm = nc.dram_tensor("mm", [P, NG * J], BF16, kind="ExternalInput")
    y = nc.dram_tensor("y", [NT * J, C], FP32, kind="ExternalOutput")

    import contextlib

    with contextlib.ExitStack() as ctx:
        mm_sb = ctx.enter_context(nc.sbuf_tensor("mm_sb", [P, NG * J], BF16))
        x_sb = ctx.enter_context(
            nc.sbuf_tensor("x_sb", [P, NT * NG * C], BF16)
        )
        y_sb = ctx.enter_context(nc.sbuf_tensor("y_sb", [NT * J, C], FP32))
        ps = ctx.enter_context(nc.psum_tensor("ps", [NT * J, C], FP32))
        mm_sem = ctx.enter_context(nc.semaphore("mm_sem"))
        # DMA completions within one HWDGE queue are NOT ordered across
        # DMAs (packets spray over 16 SDMA engines), so each chunk gets
        # its own completion semaphore.
        c_sems = [
            ctx.enter_context(nc.semaphore(f"c_sem{i}")) for i in range(NCHUNK)
        ]
        pe_sem = ctx.enter_context(nc.semaphore("pe_sem"))
        cp_sem = ctx.enter_context(nc.semaphore("cp_sem"))
        y_sem = ctx.enter_context(nc.semaphore("y_sem"))
        block = ctx.enter_context(nc.Block())

        # Even chunks + output on the sync (SP) HWDGE ring.
        @block.sync
        def _(sync):
            for c in range(0, NCHUNK, 2):
                sync.dma_start(
                    x_sb[:, c * FREE : (c + 1) * FREE],
                    x[:, c * FREE : (c + 1) * FREE],
                ).then_inc(c_sems[c], 16)
            sync.wait_ge(cp_sem, NT)
            sync.dma_start(y[:], y_sb[:]).then_inc(y_sem, 16)
            sync.wait_ge(y_sem, 16)

        # Stationary stack + odd chunks on the scalar (ACT) HWDGE ring.
        @block.scalar
        def _(scalar):
            scalar.dma_start(mm_sb[:], mm[:]).then_inc(mm_sem, 16)
            for c in range(1, NCHUNK, 2):
                scalar.dma_start(
                    x_sb[:, c * FREE : (c + 1) * FREE],
                    x[:, c * FREE : (c + 1) * FREE],
                ).then_inc(c_sems[c], 16)

        # PE chases the stream: per tile t, 8 accumulating passes g.
        @block.tensor
        def _(tensor):
            tensor.wait_ge(mm_sem, 16)
            for t in range(NT):
                for g in range(NG):
                    chunk = (t * NG + g) // PASS_PER_CHUNK
                    instr = tensor.matmul(
                        ps[t * J : (t + 1) * J, :],
                        mm_sb[:, g * J : (g + 1) * J],
                        x_sb[:, (t * NG + g) * C : (t * NG + g + 1) * C],
                        start=(g == 0),
                        stop=(g == NG - 1),
                    )
                    if g % PASS_PER_CHUNK == 0:
                        instr._wait_ge(c_sems[chunk], 16)
                    if g == NG - 1:
                        instr.then_inc(pe_sem, 1)

        # DVE drains finished PSUM tiles to SBUF.
        @block.vector
        def _(vector):
            for t in range(NT):
                vector.tensor_copy(
                    y_sb[t * J : (t + 1) * J, :],
                    ps[t * J : (t + 1) * J, :],
                )._wait_ge(pe_sem, t + 1).then_inc(cp_sem, 1)

    return nc


def kernel(**inputs: np.ndarray) -> np.ndarray:
    x = np.asarray(inputs["x"], dtype=np.float32)
    m = _collapse_weights(
        np.asarray(inputs["W_enc"], dtype=np.float32),
        np.asarray(inputs["W_temp"], dtype=np.float32),
        np.asarray(inputs["W_dec"], dtype=np.float32),
        np.asarray(inputs["W_out"], dtype=np.float32),
    )
    mm_packed = _pack_mm(m)

    nc = _build_bass()
    shard_b = B // N_CORES
    in_maps = [
        {
            "x": _pack_shard(
                x[i * shard_b : (i + 1) * shard_b].reshape(ROWS, H)
            ),
            "mm": mm_packed,
        }
        for i in range(N_CORES)
    ]
    res = run_bass_kernel_spmd(
        nc, in_maps, core_ids=list(range(N_CORES)), **RUN_KWARGS
    )
    out = []
    for r in res.results:
        ysh = np.asarray(r["y"], dtype=np.float32)  # [NT*J, C]
        # y[t*J + j, c] = token t*8192 + c*16 + j
        ysh = ysh.reshape(NT, J, C).transpose(0, 2, 1).reshape(ROWS)
        out.append(ysh.reshape(shard_b, S, 1))
    return np.concatenate(out, axis=0)


# revision 4
# speedup vs baseline: 1.6532x; 1.6532x over previous
"""Trainium2 kernel for nn_DeepLinearTimeSeries.

The reference network is a 400-layer *linear* residual MLP: every step is
x <- x @ (W_i^T) [+ 0.1 * carry], with no nonlinearities anywhere. The whole
stack therefore collapses algebraically to a single matrix:

    out = x @ m,   m = T_enc @ T_temp @ T_dec @ W_out^T  (64 x 1)

where each block's transfer matrix is the product of its per-layer factors
(W_i^T + 0.1*I), with the first two layers of the encoder/temporal blocks
handled per the reference's carry pattern (T = W0^T W1^T + 0.1 I).

We fold the 400 64x64 factors on the host (trivial FLOPs, same f32
arithmetic regime as the reference), then run the remaining memory-bound
pass y = x @ m on 8 NeuronCores, data-parallel over the batch dim
(sharding_hint). Per core: x shard [32768, 64] -> y [32768].

Device kernel (raw Bass, no Tile): x is shipped bf16 (well inside the 2e-2
tolerance; the dot is accumulated in fp32 PSUM) so the HBM stream is 4 MiB
per core instead of 8. The dot itself runs on the *tensor* engine: the host
pre-packs x so the hidden dim lives on partitions -- partition p = 4*j + d
holds dim (4g+d) of token-slot j at free offset (t*8192 + g*512 + c), token
id = t*16384 + c*32 + j. A [128 x 32] stationary slice of mm (m replicated
per dim-group g) then turns each 512-column matmul into 32-token-parallel
multiply+reduce, accumulating the g=0..15 passes into one PSUM tile
[32, 512] per t (PSUM base partitions 0/32, the PE tile-position grid).
32 matmuls total (~220 ns each) chase 8 x 512 KiB chunk DMAs that are
split across both HWDGE rings (sync + scalar) so per-DMA completion
latency hides under the other ring's data. DVE drains each finished PSUM
tile to SBUF; one 128 KiB DMA returns y.
"""

import numpy as np
from ml_dtypes import bfloat16

import concourse.bass as bass
import concourse.mybir as mybir
from concourse.bass_utils import run_bass_kernel_spmd

# Problem constants (hardcoded per harness contract).
B, S, H = 128, 2048, 64
N_CORES = 8
RW = np.float32(0.1)
ROWS = B * S // N_CORES          # 32768 tokens per core
P = 128                          # SBUF partitions
NG = 16                          # dim groups (4 dims each)
D = H // NG                      # 4 dims per group
J = P // D                       # 32 token-slots per column
C = 512                          # columns per tile (= PSUM bank free size)
NT = ROWS // (J * C)             # 2 PSUM tiles
NCHUNK = 8
FREE = NT * NG * C // NCHUNK     # 2048 free elems per partition per chunk
PASS_PER_CHUNK = FREE // C       # 4 matmul passes per chunk
FP32 = mybir.dt.float32
BF16 = mybir.dt.bfloat16

# Extra kwargs for run_bass_kernel_spmd (test harness sets these for tracing).
RUN_KWARGS: dict = {}


def _collapse_weights(W_enc, W_temp, W_dec, W_out):
    """Fold the full linear stack into a single [H, 1] f32 matrix."""
    eye = np.eye(H, dtype=np.float32)

    def block_mat(Ws):
        # x1 = x0 W0^T ; x2 = x1 W1^T + 0.1 x0 ; then x <- x (Wi^T + 0.1 I)
        T = Ws[0].T @ Ws[1].T + RW * eye
        for Wi in Ws[2:]:
            T = T @ (Wi.T + RW * eye)
        return T

    M = block_mat(W_enc) @ block_mat(W_temp)
    for Wd in W_dec:
        M = M @ (Wd.T + RW * eye)
    return (M @ W_out.T).astype(np.float32)  # [H, 1]


def _pack_shard(x_shard):
    """[32768, 64] f32 -> [128, 16384] bf16, p=(j,d), f=(t,g,c)."""
    xs = x_shard.reshape(NT, C, J, NG, D)           # [t, c, j, g, d]
    return np.ascontiguousarray(
        xs.transpose(2, 4, 0, 3, 1).reshape(P, NT * NG * C)
    ).astype(bfloat16)


def _pack_mm(m):
    """[H,1] f32 -> [128, NG*J] bf16 stationary stack (one slice per g)."""
    mm = np.zeros((P, NG * J), np.float32)
    for g in range(NG):
        for j in range(J):
            mm[D * j : D * j + D, g * J + j] = m[D * g : D * g + D, 0]
    return mm.astype(bfloat16)


def _build_bass():
    nc = bass.Bass()
    x = nc.dram_tensor("x", [P, NT * NG * C], BF16, kind="ExternalInput")
    mm = nc.dram_tensor("mm", [P, NG * J], BF16, kind="ExternalInput")
    y = nc.dram_tensor("y", [NT * J, C], FP32, kind="ExternalOutput")

    import contextlib

    with contextlib.ExitStack() as ctx:
        mm_sb = ctx.enter_context(nc.sbuf_tensor("mm_sb", [P, NG * J], BF16))
        x_sb = ctx.enter_context(
            nc.sbuf_tensor("x_sb", [P, NT * NG * C], BF16)
        )
        y_sb = ctx.enter_context(nc.sbuf_tensor("y_sb", [NT * J, C], FP32))
        ps = ctx.enter_context(nc.psum_tensor("ps", [NT * J, C], FP32))
        mm_sem = ctx.enter_context(nc.semaphore("mm_sem"))
        # DMA completions within one HWDGE queue are NOT ordered across
        # DMAs (packets spray over 16 SDMA engines), so each chunk gets
        # its own completion semaphore.
        c_sems = [
            ctx.enter_context(nc.semaphore(f"c_sem{i}")) for i in range(NCHUNK)
        ]
        pe_sem = ctx.enter_context(nc.semaphore("pe_sem"))
        cp_sem = ctx.enter_context(nc.semaphore("cp_sem"))
        y_sem = ctx.enter_context(nc.semaphore("y_sem"))
        block = ctx.enter_context(nc.Block())

        # Even chunks + output on the sync (SP) HWDGE ring.
        @block.sync
        def _(sync):
            for c in range(0, NCHUNK, 2):
                sync.dma_start(
                    x_sb[:, c * FREE : (c + 1) * FREE],
                    x[:, c * FREE : (c + 1) * FREE],
                ).then_inc(c_sems[c], 16)
            sync.wait_ge(cp_sem, NT)
            sync.dma_start(y[:], y_sb[:]).then_inc(y_sem, 16)
            sync.wait_ge(y_sem, 16)

        # Stationary stack + odd chunks on the scalar (ACT) HWDGE ring.
        @block.scalar
        def _(scalar):
            scalar.dma_start(mm_sb[:], mm[:]).then_inc(mm_sem, 16)
            for c in range(1, NCHUNK, 2):
                scalar.dma_start(
                    x_sb[:, c * FREE : (c + 1) * FREE],
                    x[:, c * FREE : (c + 1) * FREE],
                ).then_inc(c_sems[c], 16)

        # PE chases the stream: per tile t, 8 accumulating passes g.
        @block.tensor
        def _(tensor):
            tensor.wait_ge(mm_sem, 16)
            for t in range(NT):
                for g in range(NG):
                    chunk = (t * NG + g) // PASS_PER_CHUNK
                    instr = tensor.matmul(
                        ps[t * J : (t + 1) * J, :],
                        mm_sb[:, g * J : (g + 1) * J],
                        x_sb[:, (t * NG + g) * C : (t * NG + g + 1) * C],
                        start=(g == 0),
                        stop=(g == NG - 1),
                    )
                    if g % PASS_PER_CHUNK == 0:
                        instr._wait_ge(c_sems[chunk], 16)
                    if g == NG - 1:
                        instr.then_inc(pe_sem, 1)

        # DVE drains finished PSUM tiles to SBUF.
        @block.vector
        def _(vector):
            for t in range(NT):
                vector.tensor_copy(
                    y_sb[t * J : (t + 1) * J, :],
                    ps[t * J : (t + 1) * J, :],
                )._wait_ge(pe_sem, t + 1).then_inc(cp_sem, 1)

    return nc


def kernel(**inputs: np.ndarray) -> np.ndarray:
    x = np.asarray(inputs["x"], dtype=np.float32)
    m = _collapse_weights(
        np.asarray(inputs["W_enc"], dtype=np.float32),
        np.asarray(inputs["W_temp"], dtype=np.float32),
        np.asarray(inputs["W_dec"], dtype=np.float32),
        np.asarray(inputs["W_out"], dtype=np.float32),
    )
    mm_packed = _pack_mm(m)

    nc = _build_bass()
    shard_b = B // N_CORES
    in_maps = [
        {
            "x": _pack_shard(
                x[i * shard_b : (i + 1) * shard_b].reshape(ROWS, H)
            ),
            "mm": mm_packed,
        }
        for i in range(N_CORES)
    ]
    res = run_bass_kernel_spmd(
        nc, in_maps, core_ids=list(range(N_CORES)), **RUN_KWARGS
    )
    out = []
    for r in res.results:
        ysh = np.asarray(r["y"], dtype=np.float32)  # [NT*J, C]
        # y[t*J + j, c] = token t*8192 + c*16 + j
        ysh = ysh.reshape(NT, J, C).transpose(0, 2, 1).reshape(ROWS)
        out.append(ysh.reshape(shard_b, S, 1))
    return np.concatenate(out, axis=0)
